# revision 1
# baseline (speedup 1.0000x reference)
"""DSVA block (sparse dynamic voxel attention) Trainium2 kernel.

Sharding: data-parallel over (batch, half-of-voxels): 8 cores = 4 batches x 2
halves.  Each core receives its batch's FULL voxel set (candidates for KNN /
attention) rotated so that its own 2048 query voxels come first, computes the
block for those queries, and returns [2048, 64].

Device algorithm per core:
  A) LN1 folded into an augmented QKV matmul ([x*rstd, -mean*rstd, 1] @ Waug);
     kv rows ([4096, 128]) written to DRAM for the gather; q kept in SBUF.
     Candidate plane [8, 4096] = (2c, -|c|^2 - BIG*(1-mask), c, 1) built via
     PE transposes (rank(i,j) = c_i . 2c_j + pen_j = |c_i|^2 - d2_masked(i,j),
     a per-row monotone transform of -d2, so top-k matches the reference).
  B) Per 128-query block: rank via K=8 fp32 matmul into PSUM, top-10 via
     hierarchical max8 (16 chunks of 256) + match_replace, positions via two
     full-row max_index scans per half, kv gather via indirect DMA, scores,
     top-4 threshold + softmax, weighted sum of v.
  C) proj + residual (x2 = mask*(o@Wp + hb) + x), LN2-folded MLP with
     tanh-gelu composed from primitives, final = h2 + x2 + hb2.
"""

import functools
import sys

import numpy as np

sys.path.insert(0, "/opt/trn_rl_repo")

B, N, C = 4, 4096, 64
H, DH, KNN = 4, 16, 10
MLP = 256
NB = N // 128          # 32 candidate blocks
QN = N // 2            # 2048 queries per core
QB = QN // 128         # 16 query blocks
BIG = 1e9
EPS = 1e-5
NEG = -3e38
GELU_C0 = 0.044715
GELU_C1 = 0.7978845608028654  # sqrt(2/pi)


def _build_nc():
    import concourse.bass as bass
    import concourse.mybir as mybir
    import concourse.tile as tile
    from concourse import bacc
    from concourse.masks import make_identity
    from contextlib import ExitStack

    f32 = mybir.dt.float32
    u32 = mybir.dt.uint32
    A = mybir.AluOpType
    AF = mybir.ActivationFunctionType
    AX = mybir.AxisListType

    nc = bacc.Bacc()
    x_d = nc.declare_dram_parameter("x", [N, C], f32, isOutput=False)
    coords_d = nc.declare_dram_parameter("coords", [N, 3], f32, isOutput=False)
    maskc_d = nc.declare_dram_parameter("mask_cols", [128, NB], f32, isOutput=False)
    wqkv_d = nc.declare_dram_parameter("Wqkv", [C + 2, 3 * C], f32, isOutput=False)
    wp_d = nc.declare_dram_parameter("Wp", [C, C], f32, isOutput=False)
    hb_d = nc.declare_dram_parameter("hb", [C], f32, isOutput=False)
    w1_d = nc.declare_dram_parameter("W1", [C + 2, MLP], f32, isOutput=False)
    w2_d = nc.declare_dram_parameter("W2", [2, 128, C], f32, isOutput=False)
    hb2_d = nc.declare_dram_parameter("hb2", [C], f32, isOutput=False)
    out_d = nc.declare_dram_parameter("out", [QN, C], f32, isOutput=True)
    import os as _os
    _dbg = bool(int(_os.environ.get("KDBG", "0")))
    if _dbg:
        dbg_nbr = nc.declare_dram_parameter("dbg_nbr", [QN, KNN], f32, isOutput=True)
        dbg_s = nc.declare_dram_parameter("dbg_s", [QN, KNN, H], f32, isOutput=True)
        dbg_o = nc.declare_dram_parameter("dbg_o", [QN, C], f32, isOutput=True)
        dbg_q = nc.declare_dram_parameter("dbg_q", [QN, C], f32, isOutput=True)
        dbg_kv = nc.declare_dram_parameter("dbg_kv", [N, 2 * C], f32, isOutput=True)
        dbg_rank = nc.declare_dram_parameter("dbg_rank", [128, N], f32, isOutput=True)
        dbg_plane = nc.declare_dram_parameter("dbg_plane", [8, N], f32, isOutput=True)
        dbg_lhs = nc.declare_dram_parameter("dbg_lhs", [8, QN], f32, isOutput=True)
        dbg_cv = nc.declare_dram_parameter("dbg_cv", [128, 128], f32, isOutput=True)
    nbr_dram = nc.dram_tensor("nbr_scratch", [QB, KNN * 128], mybir.dt.uint16)

    def bcast_from_dram(vec_ap, p, n):
        return bass.AP(tensor=vec_ap.tensor, offset=vec_ap.offset,
                       ap=[[0, p]] + list(vec_ap.ap))

    def view(t_ap, off, dims):
        return bass.AP(tensor=t_ap.tensor, offset=t_ap.offset + off,
                       ap=[t_ap.ap[0]] + dims)

    with tile.TileContext(nc) as tc, ExitStack() as ctx:
        consts = ctx.enter_context(tc.tile_pool(name="consts", bufs=1))
        ident = consts.tile([128, 128], f32)
        make_identity(nc, ident)
        epsT = consts.tile([128, 1], f32)
        nc.vector.memset(epsT, EPS)
        plane = consts.tile([8, N], f32)
        lhsTz = consts.tile([8, QN], f32)
        q_res = consts.tile([128, QB, C], f32)
        kv_res = consts.tile([128, NB, 2 * C], mybir.dt.float16)
        iotac_i = consts.tile([128, NB], mybir.dt.int32)
        nc.gpsimd.iota(iotac_i, pattern=[[128, NB]], base=0, channel_multiplier=1)
        iotac = consts.tile([128, NB], f32)
        nc.vector.tensor_copy(out=iotac, in_=iotac_i)
        o_res = consts.tile([128, QB, C], f32)
        maskc = consts.tile([128, NB], f32)
        nc.sync.dma_start(out=maskc, in_=maskc_d[:])
        hb_b = consts.tile([128, C], f32)
        nc.sync.dma_start(out=hb_b, in_=bcast_from_dram(hb_d[:], 128, C))
        hb2_b = consts.tile([128, C], f32)
        nc.sync.dma_start(out=hb2_b, in_=bcast_from_dram(hb2_d[:], 128, C))
        wqkv_sb = consts.tile([C + 2, 3 * C], f32)
        nc.sync.dma_start(out=wqkv_sb, in_=wqkv_d[:])
        wp_sb = consts.tile([C, C], f32)
        nc.sync.dma_start(out=wp_sb, in_=wp_d[:])
        w1_sb = consts.tile([C + 2, MLP], f32)
        nc.sync.dma_start(out=w1_sb, in_=w1_d[:])
        w2_sb = consts.tile([128, 2, C], f32)
        nc.sync.dma_start(out=w2_sb, in_=w2_d[:].rearrange("k p n -> p k n"))

        # ---------------- Phase A: coords plane + qkv ----------------
        with tc.tile_pool(name="wA", bufs=3) as wa, \
             tc.tile_pool(name="psAtr", bufs=2, space="PSUM") as psatr, \
             tc.tile_pool(name="psAmm", bufs=2, space="PSUM") as psamm:
            for b in range(NB):
                cblk = wa.tile([128, 3], f32, tag="cblk")
                nc.sync.dma_start(out=cblk, in_=coords_d[128 * b:128 * (b + 1), :])
                aug = wa.tile([128, 8], f32, tag="aug")
                nc.vector.tensor_scalar_mul(aug[:, 0:3], cblk, 2.0)
                csq = wa.tile([128, 3], f32, tag="csq")
                nc.vector.tensor_tensor(out=csq, in0=cblk, in1=cblk, op=A.mult)
                sq = wa.tile([128, 1], f32, tag="sq")
                nc.vector.tensor_reduce(out=sq, in_=csq, axis=AX.X, op=A.add)
                # pen = BIG*(mask-1) - |c|^2, computed as (BIG*mask - BIG) - sq
                # so unmasked rows get exactly -sq (no 1e9 cancellation).
                pent = wa.tile([128, 1], f32, tag="pent")
                nc.vector.tensor_scalar(
                    out=pent, in0=maskc[:, b:b + 1], scalar1=float(BIG),
                    scalar2=float(-BIG), op0=A.mult, op1=A.add)
                nc.vector.tensor_sub(out=aug[:, 3:4], in0=pent, in1=sq)
                nc.vector.tensor_copy(out=aug[:, 4:7], in_=cblk)
                nc.vector.memset(aug[:, 7:8], 1.0)
                augT = psatr.tile([8, 128], f32, tag="trA")
                nc.tensor.transpose(augT, aug, ident)
                nc.scalar.copy(out=plane[:, 128 * b:128 * (b + 1)], in_=augT)
                if b < QB:
                    augq = wa.tile([128, 8], f32, tag="augq")
                    nc.vector.tensor_copy(out=augq[:, 0:3], in_=cblk)
                    nc.vector.memset(augq[:, 3:4], 1.0)
                    nc.vector.memset(augq[:, 4:8], 0.0)
                    augqT = psatr.tile([8, 128], f32, tag="trA")
                    nc.tensor.transpose(augqT, augq, ident)
                    nc.scalar.copy(out=lhsTz[:, 128 * b:128 * (b + 1)], in_=augqT)

                xb = wa.tile([128, C], f32, tag="xb")
                nc.sync.dma_start(out=xb, in_=x_d[128 * b:128 * (b + 1), :])
                st6 = wa.tile([128, 6], f32, tag="st6")
                nc.vector.bn_stats(out=st6, in_=xb)
                mv = wa.tile([128, 2], f32, tag="mv")
                nc.vector.bn_aggr(out=mv, in_=st6)
                sd = wa.tile([128, 1], f32, tag="sd")
                nc.scalar.activation(out=sd, in_=mv[:, 1:2], func=AF.Sqrt,
                                     bias=epsT, scale=1.0)
                rstd = wa.tile([128, 1], f32, tag="rstd")
                nc.vector.reciprocal(out=rstd, in_=sd)
                xa = wa.tile([128, C + 2], f32, tag="xa")
                nc.vector.tensor_scalar_mul(xa[:, 0:C], xb, rstd)
                nc.vector.tensor_scalar(
                    out=xa[:, C:C + 1], in0=mv[:, 0:1], scalar1=rstd,
                    scalar2=-1.0, op0=A.mult, op1=A.mult)
                nc.vector.memset(xa[:, C + 1:C + 2], 1.0)
                xaT_p = psatr.tile([C + 2, 128], f32, tag="trA")
                nc.tensor.transpose(xaT_p, xa, ident)
                xaT = wa.tile([C + 2, 128], f32, tag="xaT")
                nc.scalar.copy(out=xaT, in_=xaT_p)
                qkv_p = psamm.tile([128, 3 * C], f32, tag="mmq")
                nc.tensor.matmul(qkv_p, xaT, wqkv_sb, start=True, stop=True)
                if b < QB:
                    nc.scalar.copy(out=q_res[:, b, :], in_=qkv_p[:, 0:C])
                nc.scalar.copy(out=kv_res[:, b, :], in_=qkv_p[:, C:3 * C])
                if _dbg:
                    kvb = wa.tile([128, 2 * C], f32, tag="kvb")
                    nc.scalar.copy(out=kvb, in_=qkv_p[:, C:3 * C])
                    nc.sync.dma_start(out=dbg_kv[128 * b:128 * (b + 1), :], in_=kvb)


        tc.strict_bb_all_engine_barrier()

        # ---------------- Phase B: KNN + attention ----------------
        with tc.tile_pool(name="wB", bufs=3) as wb, \
             tc.tile_pool(name="kvp", bufs=3) as kvp, \
             tc.tile_pool(name="rankp", bufs=2, space="PSUM") as rkp:
            warmB = rkp.tile([128, 2048], f32, tag="rank")
            nc.vector.memset(warmB[:, 0:8], 0.0)
            warmB2 = rkp.tile([128, 2048], f32, tag="rank")
            nc.vector.memset(warmB2[:, 0:8], 0.0)
            for b in range(QB):
                lhsT = lhsTz[:, 128 * b:128 * (b + 1)]
                cv = wb.tile([128, 128], f32, tag="cv")
                rh = []
                for hf in range(2):
                    rk = rkp.tile([128, 2048], f32, tag="rank")
                    rh.append(rk)
                    for c4 in range(4):
                        nc.tensor.matmul(
                            rk[:, 512 * c4:512 * (c4 + 1)], lhsT,
                            plane[:, 2048 * hf + 512 * c4: 2048 * hf + 512 * (c4 + 1)],
                            start=True, stop=True)
                    for c8 in range(8):
                        nc.vector.max(out=cv[:, 64 * hf + 8 * c8: 64 * hf + 8 * (c8 + 1)],
                                      in_=rk[:, 256 * c8:256 * (c8 + 1)])
                if _dbg and b == 0:
                    for _hf in range(2):
                        dbg_rk_sb = wb.tile([128, 2048], f32, name=f"dbgrk{_hf}", tag=f"dbgrk{_hf}")
                        nc.scalar.copy(out=dbg_rk_sb, in_=rh[_hf])
                        nc.sync.dma_start(out=dbg_rank[:, 2048 * _hf:2048 * (_hf + 1)], in_=dbg_rk_sb)
                    nc.sync.dma_start(out=dbg_cv[:], in_=cv)
                    nc.sync.dma_start(out=dbg_plane[:], in_=plane)
                    nc.sync.dma_start(out=dbg_lhs[:], in_=lhsTz)
                t8 = wb.tile([128, 8], f32, tag="t8")
                nc.vector.max(out=t8, in_=cv)
                cvm = wb.tile([128, 128], f32, tag="cvm")
                nc.vector.match_replace(out=cvm, in_to_replace=t8, in_values=cv,
                                        imm_value=NEG)
                n8 = wb.tile([128, 8], f32, tag="n8")
                nc.vector.max(out=n8, in_=cvm)
                n2p = wb.tile([128, 8], f32, tag="n2p")
                nc.vector.memset(n2p, NEG)
                nc.vector.tensor_copy(out=n2p[:, 0:2], in_=n8[:, 0:2])

                pos = [wb.tile([128, 8], u32, tag=f"pos{k}", name=f"pos{k}_{b}") for k in range(4)]
                nc.vector.max_index(out=pos[0], in_max=t8, in_values=rh[0])
                nc.vector.max_index(out=pos[1], in_max=t8, in_values=rh[1])
                nc.vector.max_index(out=pos[2], in_max=n2p, in_values=rh[0])
                nc.vector.max_index(out=pos[3], in_max=n2p, in_values=rh[1])
                pf = [wb.tile([128, 8], f32, tag=f"pf{k}", name=f"pf{k}_{b}") for k in range(4)]
                for k in range(4):
                    nc.vector.tensor_copy(out=pf[k], in_=pos[k])
                nc.vector.tensor_scalar_add(pf[1], pf[1], 2048.0)
                nc.vector.tensor_scalar_add(pf[3], pf[3], 2048.0)
                g8 = wb.tile([128, 8], f32, tag="g8")
                nc.vector.tensor_tensor(out=g8, in0=pf[0], in1=pf[1], op=A.min)
                g2 = wb.tile([128, 8], f32, tag="g2")
                nc.vector.tensor_tensor(out=g2, in0=pf[2], in1=pf[3], op=A.min)
                id10 = wb.tile([128, KNN], f32, tag="id10")
                nc.vector.tensor_copy(out=id10[:, 0:8], in_=g8)
                nc.vector.tensor_copy(out=id10[:, 8:KNN], in_=g2[:, 0:2])
                # nbr indices -> j-major u16, broadcast to all partitions via DRAM
                idT_p = rkp.tile([KNN, 128], f32, tag="rank", name=f"idTp{b}")
                nc.tensor.transpose(idT_p, id10, ident)
                idT = wb.tile([KNN, 128], mybir.dt.uint16, tag="idT")
                nc.scalar.copy(out=idT, in_=idT_p)
                nc.sync.dma_start(out=nbr_dram[b, :].rearrange("(j i) -> j i", i=128),
                                  in_=idT)
                nbrB = wb.tile([128, KNN, 128], mybir.dt.uint16, tag="nbrB")
                nc.sync.dma_start(
                    out=nbrB,
                    in_=bass.AP(tensor=nbr_dram[:].tensor,
                                offset=nbr_dram[:].offset + b * KNN * 128,
                                ap=[[0, 128], [128, KNN], [1, 128]]))
                # one-hot gather in 3 waves (PSUM accumulation is bank-granular:
                # each j-slot accumulator owns a 512-f32 bank)
                kvn = kvp.tile([128, KNN, 2 * C], f32, tag="kvn")
                for (j0, nj) in ((0, 4), (4, 4), (8, 2)):
                    kvn_w = rkp.tile([128, 2048], f32, tag="rank",
                                     name=f"kvnw{b}_{j0}")
                    for c in range(NB):
                        pt = wb.tile([128, nj, 128], mybir.dt.float16,
                                     tag=f"pt{nj}", name=f"pt{b}_{j0}_{c}")
                        nc.vector.tensor_scalar(
                            out=pt, in0=nbrB[:, j0:j0 + nj, :],
                            scalar1=iotac[:, c:c + 1], scalar2=None,
                            op0=A.is_equal)
                        for jj in range(nj):
                            nc.tensor.matmul(
                                kvn_w[:, 512 * jj:512 * jj + 128], pt[:, jj, :],
                                kv_res[:, c, :], start=(c == 0), stop=(c == NB - 1))
                    for jj in range(nj):
                        nc.scalar.copy(out=kvn[:, j0 + jj, :],
                                       in_=kvn_w[:, 512 * jj:512 * jj + 128])

                # scores s[i, j, h] = sum_d q*kn
                prod = wb.tile([128, KNN, H, DH], f32, tag="prod")
                kn_v = kvn[:, :, 0:C].rearrange("p j (h d) -> p j h d", d=DH)
                qa = q_res[:, b, :]
                q_v = view(qa, 0, [[0, KNN], [DH, H], [1, DH]])
                nc.vector.tensor_tensor(out=prod, in0=kn_v, in1=q_v, op=A.mult)
                s = wb.tile([128, KNN, H], f32, tag="s")
                nc.vector.tensor_reduce(out=s, in_=prod, axis=AX.X, op=A.add)

                pk8 = wb.tile([128, H, 8], f32, tag="pk8")
                for h in range(H):
                    nc.vector.max(out=pk8[:, h, :], in_=s[:, :, h])
                e = wb.tile([128, KNN, H], f32, tag="e")
                nc.scalar.activation(out=e, in_=s, func=AF.Exp, bias=0.0, scale=1.0)
                s_hj = view(s[:], 0, [[1, H], [H, KNN]])
                t4_hj = view(pk8[:], 3, [[8, H], [0, KNN]])
                e_hj = view(e[:], 0, [[1, H], [H, KNN]])
                m = wb.tile([128, H, KNN], f32, tag="m")
                nc.vector.tensor_tensor(out=m, in0=s_hj, in1=t4_hj, op=A.is_ge)
                u = wb.tile([128, H, KNN], f32, tag="u")
                nc.vector.tensor_tensor(out=u, in0=m, in1=e_hj, op=A.mult)
                rs = wb.tile([128, H], f32, tag="rs")
                nc.vector.tensor_reduce(out=rs, in_=u, axis=AX.X, op=A.add)
                rc = wb.tile([128, H], f32, tag="rc")
                nc.vector.reciprocal(out=rc, in_=rs)
                wn = wb.tile([128, H, KNN], f32, tag="wn")
                rc_b = view(rc[:], 0, [[1, H], [0, KNN]])
                nc.vector.tensor_tensor(out=wn, in0=u, in1=rc_b, op=A.mult)
                prod2 = wb.tile([128, H, DH, KNN], f32, tag="p2")
                vn_hdj = view(kvn[:], C, [[DH, H], [1, DH], [2 * C, KNN]])
                wn_hdj = view(wn[:], 0, [[KNN, H], [0, DH], [1, KNN]])
                nc.vector.tensor_tensor(out=prod2, in0=vn_hdj, in1=wn_hdj, op=A.mult)
                o_v = o_res[:, b, :].rearrange("p (h d) -> p h d", d=DH)
                nc.vector.tensor_reduce(out=o_v, in_=prod2, axis=AX.X, op=A.add)
                if _dbg:
                    nc.sync.dma_start(out=dbg_nbr[128 * b:128 * (b + 1), :], in_=id10)
                    nc.sync.dma_start(out=dbg_s[128 * b:128 * (b + 1)], in_=s)
                    nc.sync.dma_start(out=dbg_o[128 * b:128 * (b + 1), :], in_=o_res[:, b, :])
                    nc.sync.dma_start(out=dbg_q[128 * b:128 * (b + 1), :], in_=q_res[:, b, :])

        tc.strict_bb_all_engine_barrier()

        # ---------------- Phase C: proj + MLP ----------------
        with tc.tile_pool(name="wC", bufs=3) as wc, \
             tc.tile_pool(name="psCtr", bufs=2, space="PSUM") as psctr, \
             tc.tile_pool(name="psCmm", bufs=2, space="PSUM") as pscmm:
            for _w in range(2):
                warmC = psctr.tile([C + 2, 128], f32, tag="trC", name=f"warmC{_w}")
                nc.vector.memset(warmC[:, 0:8], 0.0)
                warmM = pscmm.tile([128, MLP], f32, tag="mmC", name=f"warmM{_w}")
                nc.vector.memset(warmM[:, 0:8], 0.0)
            for b in range(QB):
                oT_p = psctr.tile([C, 128], f32, tag="trC")
                nc.tensor.transpose(oT_p, o_res[:, b, :], ident)
                oT = wc.tile([C, 128], f32, tag="oT")
                nc.scalar.copy(out=oT, in_=oT_p)
                a_p = pscmm.tile([128, C], f32, tag="mmC")
                nc.tensor.matmul(a_p, oT, wp_sb, start=True, stop=True)
                xb = wc.tile([128, C], f32, tag="xb2")
                nc.sync.dma_start(out=xb, in_=x_d[128 * b:128 * (b + 1), :])
                t1 = wc.tile([128, C], f32, tag="t1")
                nc.vector.tensor_add(out=t1, in0=a_p, in1=hb_b)
                x2 = wc.tile([128, C], f32, tag="x2")
                nc.vector.scalar_tensor_tensor(
                    out=x2, in0=t1, scalar=maskc[:, b:b + 1], in1=xb,
                    op0=A.mult, op1=A.add)

                st6 = wc.tile([128, 6], f32, tag="st6C")
                nc.vector.bn_stats(out=st6, in_=x2)
                mv = wc.tile([128, 2], f32, tag="mvC")
                nc.vector.bn_aggr(out=mv, in_=st6)
                sd = wc.tile([128, 1], f32, tag="sdC")
                nc.scalar.activation(out=sd, in_=mv[:, 1:2],
                                     func=mybir.ActivationFunctionType.Sqrt,
                                     bias=epsT, scale=1.0)
                rstd = wc.tile([128, 1], f32, tag="rstdC")
                nc.vector.reciprocal(out=rstd, in_=sd)
                x2a = wc.tile([128, C + 2], f32, tag="x2a")
                nc.vector.tensor_scalar_mul(x2a[:, 0:C], x2, rstd)
                nc.vector.tensor_scalar(
                    out=x2a[:, C:C + 1], in0=mv[:, 0:1], scalar1=rstd,
                    scalar2=-1.0, op0=A.mult, op1=A.mult)
                nc.vector.memset(x2a[:, C + 1:C + 2], 1.0)
                x2aT_p = psctr.tile([C + 2, 128], f32, tag="trC")
                nc.tensor.transpose(x2aT_p, x2a, ident)
                x2aT = wc.tile([C + 2, 128], f32, tag="x2aT")
                nc.scalar.copy(out=x2aT, in_=x2aT_p)
                g1_p = pscmm.tile([128, MLP], f32, tag="mmC")
                nc.tensor.matmul(g1_p, x2aT, w1_sb, start=True, stop=True)

                # gelu tanh-approx composed from primitives
                g1s = wc.tile([128, MLP], f32, tag="g1s")
                nc.scalar.copy(out=g1s, in_=g1_p)
                xg2 = wc.tile([128, MLP], f32, tag="xg2")
                nc.vector.tensor_tensor(out=xg2, in0=g1s, in1=g1s, op=A.mult)
                gco = wc.tile([128, MLP], f32, tag="gco")
                nc.gpsimd.tensor_scalar(out=gco, in0=xg2, scalar1=GELU_C0,
                                        scalar2=1.0, op0=A.mult, op1=A.add)
                gf = wc.tile([128, MLP], f32, tag="gf")
                nc.vector.tensor_tensor(out=gf, in0=gco, in1=g1s, op=A.mult)
                th = wc.tile([128, MLP], f32, tag="th")
                nc.scalar.activation(out=th, in_=gf,
                                     func=mybir.ActivationFunctionType.Tanh,
                                     bias=0.0, scale=GELU_C1)
                gr = wc.tile([128, MLP], f32, tag="gr")
                nc.gpsimd.tensor_scalar(out=gr, in0=th, scalar1=0.5,
                                        scalar2=0.5, op0=A.mult, op1=A.add)
                gl = wc.tile([128, MLP], f32, tag="gl")
                nc.vector.tensor_tensor(out=gl, in0=gr, in1=g1s, op=A.mult)

                glT = []
                for k in range(2):
                    glT_p = psctr.tile([128, 128], f32, tag="trC")
                    nc.tensor.transpose(glT_p, gl[:, 128 * k:128 * (k + 1)], ident)
                    glTk = wc.tile([128, 128], f32, tag=f"glT{k}")
                    nc.scalar.copy(out=glTk, in_=glT_p)
                    glT.append(glTk)
                h2_p = pscmm.tile([128, C], f32, tag="mmC")
                nc.tensor.matmul(h2_p, glT[0], w2_sb[:, 0, :], start=True, stop=False)
                nc.tensor.matmul(h2_p, glT[1], w2_sb[:, 1, :], start=False, stop=True)
                x2b = wc.tile([128, C], f32, tag="x2b")
                nc.vector.tensor_add(out=x2b, in0=x2, in1=hb2_b)
                fin = wc.tile([128, C], f32, tag="fin")
                nc.vector.tensor_add(out=fin, in0=h2_p, in1=x2b)
                nc.sync.dma_start(out=out_d[128 * b:128 * (b + 1), :], in_=fin)

    nc.compile()
    return nc


@functools.cache
def _get_nc():
    return _build_nc()


def _prepare_weights(ln1_g, ln1_b, ln2_g, ln2_b, w_qkv, b_qkv, w_proj, b_proj,
                     w1, b1, w2, b2):
    f = np.float32
    ln1_g, ln1_b = np.asarray(ln1_g, f), np.asarray(ln1_b, f)
    ln2_g, ln2_b = np.asarray(ln2_g, f), np.asarray(ln2_b, f)
    w_qkv, b_qkv = np.asarray(w_qkv, f), np.asarray(b_qkv, f)
    w_proj, b_proj = np.asarray(w_proj, f), np.asarray(b_proj, f)
    w1, b1 = np.asarray(w1, f), np.asarray(b1, f)
    w2, b2 = np.asarray(w2, f), np.asarray(b2, f)

    Wq = ln1_g[:, None] * w_qkv
    u = ln1_g @ w_qkv
    vc = ln1_b @ w_qkv + b_qkv
    Waug = np.concatenate([Wq, u[None], vc[None]], 0).astype(f)
    Waug[:, 0:C] *= f(1.0 / np.sqrt(DH))      # fold attention scale into q

    Wp = (0.5 * w_proj).astype(f)
    hb = (0.5 * b_proj).astype(f)
    W1g = ln2_g[:, None] * w1
    u1 = ln2_g @ w1
    v1 = ln2_b @ w1 + b1
    W1aug = np.concatenate([W1g, u1[None], v1[None]], 0).astype(f)
    W2r = np.ascontiguousarray((0.5 * w2).reshape(2, 128, C)).astype(f)
    hb2 = (0.5 * b2).astype(f)
    return dict(Wqkv=Waug, Wp=Wp, hb=hb, W1=W1aug, W2=W2r, hb2=hb2)


def _prepare_in_maps(inputs, voxel_coords, non_empty_mask, **weights):
    f = np.float32
    x = np.asarray(inputs, f)
    coords = np.asarray(voxel_coords, f)
    mask = np.asarray(non_empty_mask)
    wmaps = _prepare_weights(**weights)
    in_maps = []
    for c in range(8):
        bi, hf = c // 2, c % 2
        rot = lambda a: np.ascontiguousarray(np.roll(a, -QN * hf, axis=0))
        mr = rot(mask[bi]).astype(f)
        in_maps.append(dict(
            x=rot(x[bi]),
            coords=rot(coords[bi]),
            mask_cols=np.ascontiguousarray(mr.reshape(NB, 128).T),
            **wmaps,
        ))
    return in_maps


def _assemble(results):
    out = np.empty((B, N, C), np.float32)
    for c in range(8):
        bi, hf = c // 2, c % 2
        out[bi, QN * hf:QN * (hf + 1)] = results[c]["out"]
    return out


def kernel(inputs, voxel_coords, non_empty_mask, ln1_g, ln1_b, ln2_g, ln2_b,
           w_qkv, b_qkv, w_proj, b_proj, w1, b1, w2, b2):
    from concourse.bass_utils import run_bass_kernel_spmd

    nc = _get_nc()
    in_maps = _prepare_in_maps(
        inputs, voxel_coords, non_empty_mask,
        ln1_g=ln1_g, ln1_b=ln1_b, ln2_g=ln2_g, ln2_b=ln2_b,
        w_qkv=w_qkv, b_qkv=b_qkv, w_proj=w_proj, b_proj=b_proj,
        w1=w1, b1=b1, w2=w2, b2=b2)
    res = run_bass_kernel_spmd(nc, in_maps, list(range(8)))
    return _assemble(res.results)



# revision 3
# speedup vs baseline: 1.6696x; 1.6696x over previous
"""DSVA block (sparse dynamic voxel attention) Trainium2 kernel.

Sharding: data-parallel over (batch, half-of-voxels): 8 cores = 4 batches x 2
halves.  Each core receives its batch's FULL voxel set (candidates for KNN /
attention) rotated so that its own 2048 query voxels come first, computes the
block for those queries, and returns [2048, 64].

Device algorithm per core:
  A) LN1 folded into an augmented QKV matmul ([x*rstd, -mean*rstd, 1] @ Waug);
     kv rows ([4096, 128] f32) written to DRAM for the indirect gather; q kept
     in SBUF.  Candidate plane [8, 4096] = (2c, -|c|^2 - BIG*(1-mask), c, 1)
     built via PE transposes (rank(i,j) = c_i . 2c_j + pen_j = |c_i|^2 -
     d2_masked(i,j), a per-row monotone transform of -d2, so top-k matches the
     reference).
  B) Per 128-query block: rank via K=8 fp32 matmul into PSUM, top-10 via
     hierarchical max8 (16 chunks of 256) + match_replace, positions via two
     full-row max_index scans per half, kv gather via 10 indirect DMAs (one
     row per partition per call), scores, top-4 threshold + softmax, weighted
     sum of v.
  C) proj + residual (x2 = mask*(o@Wp + hb) + x), LN2-folded MLP with a single
     fused Gelu_apprx_tanh activation, final = h2 + x2 + hb2.
"""

import functools
import sys

import numpy as np

sys.path.insert(0, "/opt/trn_rl_repo")

B, N, C = 4, 4096, 64
H, DH, KNN = 4, 16, 10
MLP = 256
NB = N // 128          # 32 candidate blocks
QN = N // 2            # 2048 queries per core
QB = QN // 128         # 16 query blocks
BIG = 1e9
EPS = 1e-5
NEG = -3e38


def _build_nc():
    import concourse.bass as bass
    import concourse.mybir as mybir
    import concourse.tile as tile
    from concourse import bacc
    from concourse.masks import make_identity
    from contextlib import ExitStack

    f32 = mybir.dt.float32
    u32 = mybir.dt.uint32
    i32 = mybir.dt.int32
    A = mybir.AluOpType
    AF = mybir.ActivationFunctionType
    AX = mybir.AxisListType

    nc = bacc.Bacc()
    x_d = nc.declare_dram_parameter("x", [N, C], f32, isOutput=False)
    coords_d = nc.declare_dram_parameter("coords", [N, 3], f32, isOutput=False)
    maskc_d = nc.declare_dram_parameter("mask_cols", [128, NB], f32, isOutput=False)
    wqkv_d = nc.declare_dram_parameter("Wqkv", [C + 2, 3 * C], f32, isOutput=False)
    wp_d = nc.declare_dram_parameter("Wp", [C, C], f32, isOutput=False)
    hb_d = nc.declare_dram_parameter("hb", [C], f32, isOutput=False)
    w1_d = nc.declare_dram_parameter("W1", [C + 2, MLP], f32, isOutput=False)
    w2_d = nc.declare_dram_parameter("W2", [2, 128, C], f32, isOutput=False)
    hb2_d = nc.declare_dram_parameter("hb2", [C], f32, isOutput=False)
    out_d = nc.declare_dram_parameter("out", [QN, C], f32, isOutput=True)
    kv_dram = nc.dram_tensor("kv_rows", [N, 2 * C], f32)

    def bcast_from_dram(vec_ap, p, n):
        return bass.AP(tensor=vec_ap.tensor, offset=vec_ap.offset,
                       ap=[[0, p]] + list(vec_ap.ap))

    def view(t_ap, off, dims):
        return bass.AP(tensor=t_ap.tensor, offset=t_ap.offset + off,
                       ap=[t_ap.ap[0]] + dims)

    with tile.TileContext(nc) as tc, ExitStack() as ctx:
        consts = ctx.enter_context(tc.tile_pool(name="consts", bufs=1))
        ident = consts.tile([128, 128], f32)
        make_identity(nc, ident)
        epsT = consts.tile([128, 1], f32)
        nc.vector.memset(epsT, EPS)
        plane = consts.tile([8, N], f32)
        lhsTz = consts.tile([8, QN], f32)
        q_res = consts.tile([128, QB, C], f32)
        o_res = consts.tile([128, QB, C], f32)
        maskc = consts.tile([128, NB], f32)
        nc.sync.dma_start(out=maskc, in_=maskc_d[:])
        hb_b = consts.tile([128, C], f32)
        nc.sync.dma_start(out=hb_b, in_=bcast_from_dram(hb_d[:], 128, C))
        hb2_b = consts.tile([128, C], f32)
        nc.sync.dma_start(out=hb2_b, in_=bcast_from_dram(hb2_d[:], 128, C))
        wqkv_sb = consts.tile([C + 2, 3 * C], f32)
        nc.sync.dma_start(out=wqkv_sb, in_=wqkv_d[:])
        wp_sb = consts.tile([C, C], f32)
        nc.sync.dma_start(out=wp_sb, in_=wp_d[:])
        w1_sb = consts.tile([C + 2, MLP], f32)
        nc.sync.dma_start(out=w1_sb, in_=w1_d[:])
        w2_sb = consts.tile([128, 2, C], f32)
        nc.sync.dma_start(out=w2_sb, in_=w2_d[:].rearrange("k p n -> p k n"))

        # ---------------- Phase A: coords plane + qkv ----------------
        with tc.tile_pool(name="wA", bufs=3) as wa, \
             tc.tile_pool(name="psAtr", bufs=2, space="PSUM") as psatr, \
             tc.tile_pool(name="psAmm", bufs=2, space="PSUM") as psamm:
            for b in range(NB):
                cblk = wa.tile([128, 3], f32, tag="cblk")
                nc.sync.dma_start(out=cblk, in_=coords_d[128 * b:128 * (b + 1), :])
                aug = wa.tile([128, 8], f32, tag="aug")
                nc.vector.tensor_scalar_mul(aug[:, 0:3], cblk, 2.0)
                csq = wa.tile([128, 3], f32, tag="csq")
                nc.vector.tensor_tensor(out=csq, in0=cblk, in1=cblk, op=A.mult)
                sq = wa.tile([128, 1], f32, tag="sq")
                nc.vector.tensor_reduce(out=sq, in_=csq, axis=AX.X, op=A.add)
                # pen = BIG*(mask-1) - |c|^2, computed as (BIG*mask - BIG) - sq
                # so unmasked rows get exactly -sq (no 1e9 cancellation).
                pent = wa.tile([128, 1], f32, tag="pent")
                nc.vector.tensor_scalar(
                    out=pent, in0=maskc[:, b:b + 1], scalar1=float(BIG),
                    scalar2=float(-BIG), op0=A.mult, op1=A.add)
                nc.vector.tensor_sub(out=aug[:, 3:4], in0=pent, in1=sq)
                nc.vector.tensor_copy(out=aug[:, 4:7], in_=cblk)
                nc.vector.memset(aug[:, 7:8], 1.0)
                augT = psatr.tile([8, 128], f32, tag="trA")
                nc.tensor.transpose(augT, aug, ident)
                nc.scalar.copy(out=plane[:, 128 * b:128 * (b + 1)], in_=augT)
                if b < QB:
                    augq = wa.tile([128, 8], f32, tag="augq")
                    nc.vector.tensor_copy(out=augq[:, 0:3], in_=cblk)
                    nc.vector.memset(augq[:, 3:4], 1.0)
                    nc.vector.memset(augq[:, 4:8], 0.0)
                    augqT = psatr.tile([8, 128], f32, tag="trA")
                    nc.tensor.transpose(augqT, augq, ident)
                    nc.scalar.copy(out=lhsTz[:, 128 * b:128 * (b + 1)], in_=augqT)

                xb = wa.tile([128, C], f32, tag="xb")
                nc.sync.dma_start(out=xb, in_=x_d[128 * b:128 * (b + 1), :])
                st6 = wa.tile([128, 6], f32, tag="st6")
                nc.vector.bn_stats(out=st6, in_=xb)
                mv = wa.tile([128, 2], f32, tag="mv")
                nc.vector.bn_aggr(out=mv, in_=st6)
                sd = wa.tile([128, 1], f32, tag="sd")
                nc.scalar.activation(out=sd, in_=mv[:, 1:2], func=AF.Sqrt,
                                     bias=epsT, scale=1.0)
                rstd = wa.tile([128, 1], f32, tag="rstd")
                nc.vector.reciprocal(out=rstd, in_=sd)
                xa = wa.tile([128, C + 2], f32, tag="xa")
                nc.vector.tensor_scalar_mul(xa[:, 0:C], xb, rstd)
                nc.vector.tensor_scalar(
                    out=xa[:, C:C + 1], in0=mv[:, 0:1], scalar1=rstd,
                    scalar2=-1.0, op0=A.mult, op1=A.mult)
                nc.vector.memset(xa[:, C + 1:C + 2], 1.0)
                xaT_p = psatr.tile([C + 2, 128], f32, tag="trA")
                nc.tensor.transpose(xaT_p, xa, ident)
                xaT = wa.tile([C + 2, 128], f32, tag="xaT")
                nc.scalar.copy(out=xaT, in_=xaT_p)
                qkv_p = psamm.tile([128, 3 * C], f32, tag="mmq")
                nc.tensor.matmul(qkv_p, xaT, wqkv_sb, start=True, stop=True)
                if b < QB:
                    nc.scalar.copy(out=q_res[:, b, :], in_=qkv_p[:, 0:C])
                kvb = wa.tile([128, 2 * C], f32, tag="kvb")
                nc.scalar.copy(out=kvb, in_=qkv_p[:, C:3 * C])
                nc.sync.dma_start(out=kv_dram[128 * b:128 * (b + 1), :], in_=kvb)

        tc.strict_bb_all_engine_barrier()

        # ---------------- Phase B: KNN + attention ----------------
        with tc.tile_pool(name="wB", bufs=3) as wb, \
             tc.tile_pool(name="kvp", bufs=3) as kvp, \
             tc.tile_pool(name="rankp", bufs=2, space="PSUM") as rkp:
            warmB = rkp.tile([128, 2048], f32, tag="rank")
            nc.vector.memset(warmB[:, 0:8], 0.0)
            warmB2 = rkp.tile([128, 2048], f32, tag="rank")
            nc.vector.memset(warmB2[:, 0:8], 0.0)
            for b in range(QB):
                lhsT = lhsTz[:, 128 * b:128 * (b + 1)]
                cv = wb.tile([128, 128], f32, tag="cv")
                rh = []
                for hf in range(2):
                    rk = rkp.tile([128, 2048], f32, tag="rank")
                    rh.append(rk)
                    for c4 in range(4):
                        nc.tensor.matmul(
                            rk[:, 512 * c4:512 * (c4 + 1)], lhsT,
                            plane[:, 2048 * hf + 512 * c4: 2048 * hf + 512 * (c4 + 1)],
                            start=True, stop=True)
                    for c8 in range(8):
                        nc.vector.max(out=cv[:, 64 * hf + 8 * c8: 64 * hf + 8 * (c8 + 1)],
                                      in_=rk[:, 256 * c8:256 * (c8 + 1)])
                t8 = wb.tile([128, 8], f32, tag="t8")
                nc.vector.max(out=t8, in_=cv)
                cvm = wb.tile([128, 128], f32, tag="cvm")
                nc.vector.match_replace(out=cvm, in_to_replace=t8, in_values=cv,
                                        imm_value=NEG)
                n8 = wb.tile([128, 8], f32, tag="n8")
                nc.vector.max(out=n8, in_=cvm)
                n2p = wb.tile([128, 8], f32, tag="n2p")
                nc.vector.memset(n2p, NEG)
                nc.vector.tensor_copy(out=n2p[:, 0:2], in_=n8[:, 0:2])

                pos = [wb.tile([128, 8], u32, tag=f"pos{k}", name=f"pos{k}_{b}") for k in range(4)]
                nc.vector.max_index(out=pos[0], in_max=t8, in_values=rh[0])
                nc.vector.max_index(out=pos[1], in_max=t8, in_values=rh[1])
                nc.vector.max_index(out=pos[2], in_max=n2p, in_values=rh[0])
                nc.vector.max_index(out=pos[3], in_max=n2p, in_values=rh[1])
                pf = [wb.tile([128, 8], f32, tag=f"pf{k}", name=f"pf{k}_{b}") for k in range(4)]
                for k in range(4):
                    nc.vector.tensor_copy(out=pf[k], in_=pos[k])
                nc.vector.tensor_scalar_add(pf[1], pf[1], 2048.0)
                nc.vector.tensor_scalar_add(pf[3], pf[3], 2048.0)
                g8 = wb.tile([128, 8], f32, tag="g8")
                nc.vector.tensor_tensor(out=g8, in0=pf[0], in1=pf[1], op=A.min)
                g2 = wb.tile([128, 8], f32, tag="g2")
                nc.vector.tensor_tensor(out=g2, in0=pf[2], in1=pf[3], op=A.min)
                id10 = wb.tile([128, KNN], f32, tag="id10")
                nc.vector.tensor_copy(out=id10[:, 0:8], in_=g8)
                nc.vector.tensor_copy(out=id10[:, 8:KNN], in_=g2[:, 0:2])
                idx32 = wb.tile([128, KNN], i32, tag="idx32")
                nc.vector.tensor_copy(out=idx32, in_=id10)

                # kv gather: one indirect DMA per neighbor slot (one row per
                # partition per call)
                kvn = kvp.tile([128, KNN, 2 * C], f32, tag="kvn")
                for j in range(KNN):
                    nc.gpsimd.indirect_dma_start(
                        out=kvn[:, j, :],
                        out_offset=None,
                        in_=kv_dram[:],
                        in_offset=bass.IndirectOffsetOnAxis(
                            ap=idx32[:, j:j + 1], axis=0),
                    )

                # scores s[i, j, h] = sum_d q*kn
                prod = wb.tile([128, KNN, H, DH], f32, tag="prod")
                kn_v = kvn[:, :, 0:C].rearrange("p j (h d) -> p j h d", d=DH)
                qa = q_res[:, b, :]
                q_v = view(qa, 0, [[0, KNN], [DH, H], [1, DH]])
                nc.vector.tensor_tensor(out=prod, in0=kn_v, in1=q_v, op=A.mult)
                s = wb.tile([128, KNN, H], f32, tag="s")
                nc.vector.tensor_reduce(out=s, in_=prod, axis=AX.X, op=A.add)

                pk8 = wb.tile([128, H, 8], f32, tag="pk8")
                for h in range(H):
                    nc.vector.max(out=pk8[:, h, :], in_=s[:, :, h])
                e = wb.tile([128, KNN, H], f32, tag="e")
                nc.scalar.activation(out=e, in_=s, func=AF.Exp, bias=0.0, scale=1.0)
                s_hj = view(s[:], 0, [[1, H], [H, KNN]])
                t4_hj = view(pk8[:], 3, [[8, H], [0, KNN]])
                e_hj = view(e[:], 0, [[1, H], [H, KNN]])
                m = wb.tile([128, H, KNN], f32, tag="m")
                nc.vector.tensor_tensor(out=m, in0=s_hj, in1=t4_hj, op=A.is_ge)
                u = wb.tile([128, H, KNN], f32, tag="u")
                nc.vector.tensor_tensor(out=u, in0=m, in1=e_hj, op=A.mult)
                rs = wb.tile([128, H], f32, tag="rs")
                nc.vector.tensor_reduce(out=rs, in_=u, axis=AX.X, op=A.add)
                rc = wb.tile([128, H], f32, tag="rc")
                nc.vector.reciprocal(out=rc, in_=rs)
                wn = wb.tile([128, H, KNN], f32, tag="wn")
                rc_b = view(rc[:], 0, [[1, H], [0, KNN]])
                nc.vector.tensor_tensor(out=wn, in0=u, in1=rc_b, op=A.mult)
                prod2 = wb.tile([128, H, DH, KNN], f32, tag="p2")
                vn_hdj = view(kvn[:], C, [[DH, H], [1, DH], [2 * C, KNN]])
                wn_hdj = view(wn[:], 0, [[KNN, H], [0, DH], [1, KNN]])
                nc.vector.tensor_tensor(out=prod2, in0=vn_hdj, in1=wn_hdj, op=A.mult)
                o_v = o_res[:, b, :].rearrange("p (h d) -> p h d", d=DH)
                nc.vector.tensor_reduce(out=o_v, in_=prod2, axis=AX.X, op=A.add)

        tc.strict_bb_all_engine_barrier()

        # ---------------- Phase C: proj + MLP ----------------
        with tc.tile_pool(name="wC", bufs=4) as wc, \
             tc.tile_pool(name="psCtr", bufs=4, space="PSUM") as psctr, \
             tc.tile_pool(name="psCmm", bufs=4, space="PSUM") as pscmm:
            for _w in range(4):
                warmC = psctr.tile([C + 2, 128], f32, tag="trC", name=f"warmC{_w}")
                nc.vector.memset(warmC[:, 0:8], 0.0)
                warmM = pscmm.tile([128, MLP], f32, tag="mmC", name=f"warmM{_w}")
                nc.vector.memset(warmM[:, 0:8], 0.0)
            for b in range(QB):
                oT_p = psctr.tile([C, 128], f32, tag="trC")
                nc.tensor.transpose(oT_p, o_res[:, b, :], ident)
                oT = wc.tile([C, 128], f32, tag="oT")
                nc.scalar.copy(out=oT, in_=oT_p)
                a_p = pscmm.tile([128, C], f32, tag="mmC")
                nc.tensor.matmul(a_p, oT, wp_sb, start=True, stop=True)
                xb = wc.tile([128, C], f32, tag="xb2")
                nc.sync.dma_start(out=xb, in_=x_d[128 * b:128 * (b + 1), :])
                t1 = wc.tile([128, C], f32, tag="t1")
                nc.vector.tensor_add(out=t1, in0=a_p, in1=hb_b)
                x2 = wc.tile([128, C], f32, tag="x2")
                nc.vector.scalar_tensor_tensor(
                    out=x2, in0=t1, scalar=maskc[:, b:b + 1], in1=xb,
                    op0=A.mult, op1=A.add)

                st6 = wc.tile([128, 6], f32, tag="st6C")
                nc.vector.bn_stats(out=st6, in_=x2)
                mv = wc.tile([128, 2], f32, tag="mvC")
                nc.vector.bn_aggr(out=mv, in_=st6)
                sd = wc.tile([128, 1], f32, tag="sdC")
                nc.scalar.activation(out=sd, in_=mv[:, 1:2],
                                     func=mybir.ActivationFunctionType.Sqrt,
                                     bias=epsT, scale=1.0)
                rstd = wc.tile([128, 1], f32, tag="rstdC")
                nc.vector.reciprocal(out=rstd, in_=sd)
                x2a = wc.tile([128, C + 2], f32, tag="x2a")
                nc.vector.tensor_scalar_mul(x2a[:, 0:C], x2, rstd)
                nc.vector.tensor_scalar(
                    out=x2a[:, C:C + 1], in0=mv[:, 0:1], scalar1=rstd,
                    scalar2=-1.0, op0=A.mult, op1=A.mult)
                nc.vector.memset(x2a[:, C + 1:C + 2], 1.0)
                x2aT_p = psctr.tile([C + 2, 128], f32, tag="trC")
                nc.tensor.transpose(x2aT_p, x2a, ident)
                x2aT = wc.tile([C + 2, 128], f32, tag="x2aT")
                nc.scalar.copy(out=x2aT, in_=x2aT_p)
                g1_p = pscmm.tile([128, MLP], f32, tag="mmC")
                nc.tensor.matmul(g1_p, x2aT, w1_sb, start=True, stop=True)

                gl = wc.tile([128, MLP], f32, tag="gl")
                nc.scalar.activation(out=gl, in_=g1_p, func=AF.Gelu_apprx_tanh,
                                     bias=0.0, scale=1.0)

                glT = []
                for k in range(2):
                    glT_p = psctr.tile([128, 128], f32, tag="trC")
                    nc.tensor.transpose(glT_p, gl[:, 128 * k:128 * (k + 1)], ident)
                    glTk = wc.tile([128, 128], f32, tag=f"glT{k}")
                    nc.scalar.copy(out=glTk, in_=glT_p)
                    glT.append(glTk)
                h2_p = pscmm.tile([128, C], f32, tag="mmC")
                nc.tensor.matmul(h2_p, glT[0], w2_sb[:, 0, :], start=True, stop=False)
                nc.tensor.matmul(h2_p, glT[1], w2_sb[:, 1, :], start=False, stop=True)
                x2b = wc.tile([128, C], f32, tag="x2b")
                nc.vector.tensor_add(out=x2b, in0=x2, in1=hb2_b)
                fin = wc.tile([128, C], f32, tag="fin")
                nc.vector.tensor_add(out=fin, in0=h2_p, in1=x2b)
                nc.sync.dma_start(out=out_d[128 * b:128 * (b + 1), :], in_=fin)

    nc.compile()
    return nc


@functools.cache
def _get_nc():
    return _build_nc()


def _prepare_weights(ln1_g, ln1_b, ln2_g, ln2_b, w_qkv, b_qkv, w_proj, b_proj,
                     w1, b1, w2, b2):
    f = np.float32
    ln1_g, ln1_b = np.asarray(ln1_g, f), np.asarray(ln1_b, f)
    ln2_g, ln2_b = np.asarray(ln2_g, f), np.asarray(ln2_b, f)
    w_qkv, b_qkv = np.asarray(w_qkv, f), np.asarray(b_qkv, f)
    w_proj, b_proj = np.asarray(w_proj, f), np.asarray(b_proj, f)
    w1, b1 = np.asarray(w1, f), np.asarray(b1, f)
    w2, b2 = np.asarray(w2, f), np.asarray(b2, f)

    Wq = ln1_g[:, None] * w_qkv
    u = ln1_g @ w_qkv
    vc = ln1_b @ w_qkv + b_qkv
    Waug = np.concatenate([Wq, u[None], vc[None]], 0).astype(f)
    Waug[:, 0:C] *= f(1.0 / np.sqrt(DH))      # fold attention scale into q
    Wp = (0.5 * w_proj).astype(f)
    hb = (0.5 * b_proj).astype(f)
    W1g = ln2_g[:, None] * w1
    u1 = ln2_g @ w1
    v1 = ln2_b @ w1 + b1
    W1aug = np.concatenate([W1g, u1[None], v1[None]], 0).astype(f)
    W2r = np.ascontiguousarray((0.5 * w2).reshape(2, 128, C)).astype(f)
    hb2 = (0.5 * b2).astype(f)
    return dict(Wqkv=Waug, Wp=Wp, hb=hb, W1=W1aug, W2=W2r, hb2=hb2)


def _prepare_in_maps(inputs, voxel_coords, non_empty_mask, **weights):
    f = np.float32
    x = np.asarray(inputs, f)
    coords = np.asarray(voxel_coords, f)
    mask = np.asarray(non_empty_mask)
    wmaps = _prepare_weights(**weights)
    in_maps = []
    for c in range(8):
        bi, hf = c // 2, c % 2
        rot = lambda a: np.ascontiguousarray(np.roll(a, -QN * hf, axis=0))
        mr = rot(mask[bi]).astype(f)
        in_maps.append(dict(
            x=rot(x[bi]),
            coords=rot(coords[bi]),
            mask_cols=np.ascontiguousarray(mr.reshape(NB, 128).T),
            **wmaps,
        ))
    return in_maps


def _assemble(results):
    out = np.empty((B, N, C), np.float32)
    for c in range(8):
        bi, hf = c // 2, c % 2
        out[bi, QN * hf:QN * (hf + 1)] = results[c]["out"]
    return out


def kernel(inputs, voxel_coords, non_empty_mask, ln1_g, ln1_b, ln2_g, ln2_b,
           w_qkv, b_qkv, w_proj, b_proj, w1, b1, w2, b2):
    from concourse.bass_utils import run_bass_kernel_spmd

    nc = _get_nc()
    in_maps = _prepare_in_maps(
        inputs, voxel_coords, non_empty_mask,
        ln1_g=ln1_g, ln1_b=ln1_b, ln2_g=ln2_g, ln2_b=ln2_b,
        w_qkv=w_qkv, b_qkv=b_qkv, w_proj=w_proj, b_proj=b_proj,
        w1=w1, b1=b1, w2=w2, b2=b2)
    res = run_bass_kernel_spmd(nc, in_maps, list(range(8)))
    return _assemble(res.results)


# revision 10
# speedup vs baseline: 1.7833x; 1.0681x over previous
"""DSVA block (sparse dynamic voxel attention) Trainium2 kernel.

Sharding: data-parallel over (batch, half-of-voxels): 8 cores = 4 batches x 2
halves.  Each core receives its batch's FULL voxel set (candidates for KNN /
attention) rotated so that its own 2048 query voxels come first, computes the
block for those queries, and returns [2048, 64].

Device algorithm per core:
  A) LN1 folded into an augmented QKV matmul ([x*rstd, -mean*rstd, 1] @ Waug);
     kv rows ([4096, 128] f32) written to DRAM for the indirect gather; q kept
     in SBUF.  Candidate plane [8, 4096] = (2c, -|c|^2 - BIG*(1-mask), c, 1)
     built via PE transposes (rank(i,j) = c_i . 2c_j + pen_j = |c_i|^2 -
     d2_masked(i,j), a per-row monotone transform of -d2, so top-k matches the
     reference).
  B+C fused per 128-query block:
     rank via K=8 fp32r matmuls into PSUM quarters (2-buffer ping-pong, 4
     banks), quarter results copied to SBUF by the scalar engine; top-10 via
     hierarchical max8 over PSUM quarters + match_replace, positions via two
     full-row max_index scans over the SBUF copy; kv gather via 10 indirect
     DMAs (one row per partition per call); scores + top-4 softmax split
     between gpsimd and vector; then proj + residual + LN2-folded MLP.
     Scalar-engine functions in this phase are only {exp, ln, square, copy}
     (one activation table): rstd = exp(-0.5*ln(var+eps)), gelu(x) =
     x*sigmoid(2*z) computed with square+exp.
"""

import functools
import sys

import numpy as np

sys.path.insert(0, "/opt/trn_rl_repo")

B, N, C = 4, 4096, 64
H, DH, KNN = 4, 16, 10
MLP = 256
NB = N // 128          # 32 candidate blocks
QN = N // 2            # 2048 queries per core
QB = QN // 128         # 16 query blocks
BIG = 1e9
EPS = 1e-5
NEG = -3e38
GELU_C0 = 0.044715
GELU_C1 = 0.7978845608028654  # sqrt(2/pi)


def _build_nc():
    import concourse.bass as bass
    import concourse.mybir as mybir
    import concourse.tile as tile
    from concourse import bacc
    from concourse.masks import make_identity
    from contextlib import ExitStack

    f32 = mybir.dt.float32
    f32r = mybir.dt.float32r
    u32 = mybir.dt.uint32
    i32 = mybir.dt.int32
    A = mybir.AluOpType
    AF = mybir.ActivationFunctionType
    AX = mybir.AxisListType

    nc = bacc.Bacc()
    x_d = nc.declare_dram_parameter("x", [N, C], f32, isOutput=False)
    coords_d = nc.declare_dram_parameter("coords", [N, 3], f32, isOutput=False)
    maskc_d = nc.declare_dram_parameter("mask_cols", [128, NB], f32, isOutput=False)
    wqkv_d = nc.declare_dram_parameter("Wqkv", [C + 2, 3 * C], f32, isOutput=False)
    wp_d = nc.declare_dram_parameter("Wp", [C, C], f32, isOutput=False)
    hb_d = nc.declare_dram_parameter("hb", [C], f32, isOutput=False)
    w1_d = nc.declare_dram_parameter("W1", [C + 2, MLP], f32, isOutput=False)
    w2_d = nc.declare_dram_parameter("W2", [2, 128, C], f32, isOutput=False)
    hb2_d = nc.declare_dram_parameter("hb2", [C], f32, isOutput=False)
    out_d = nc.declare_dram_parameter("out", [QN, C], f32, isOutput=True)
    kv_dram = nc.dram_tensor("kv_rows", [N, 2 * C], f32)

    def bcast_from_dram(vec_ap, p, n):
        return bass.AP(tensor=vec_ap.tensor, offset=vec_ap.offset,
                       ap=[[0, p]] + list(vec_ap.ap))

    def view(t_ap, off, dims):
        return bass.AP(tensor=t_ap.tensor, offset=t_ap.offset + off,
                       ap=[t_ap.ap[0]] + dims)

    def r(ap):
        return ap.bitcast(f32r)

    with tile.TileContext(nc) as tc, ExitStack() as ctx:
        consts = ctx.enter_context(tc.tile_pool(name="consts", bufs=1))
        ident = consts.tile([128, 128], f32)
        make_identity(nc, ident)
        epsT = consts.tile([128, 1], f32)
        nc.vector.memset(epsT, EPS)
        plane = consts.tile([8, N], f32)
        lhsTz = consts.tile([8, QN], f32)
        q_res = consts.tile([128, QB, C], f32)
        maskc = consts.tile([128, NB], f32)
        nc.sync.dma_start(out=maskc, in_=maskc_d[:])
        hb_b = consts.tile([128, C], f32)
        nc.sync.dma_start(out=hb_b, in_=bcast_from_dram(hb_d[:], 128, C))
        hb2_b = consts.tile([128, C], f32)
        nc.sync.dma_start(out=hb2_b, in_=bcast_from_dram(hb2_d[:], 128, C))
        wqkv_sb = consts.tile([C + 2, 3 * C], f32)
        nc.sync.dma_start(out=wqkv_sb, in_=wqkv_d[:])
        wp_sb = consts.tile([C, C], f32)
        nc.sync.dma_start(out=wp_sb, in_=wp_d[:])
        w1_sb = consts.tile([C + 2, MLP], f32)
        nc.sync.dma_start(out=w1_sb, in_=w1_d[:])
        w2_sb = consts.tile([128, 2, C], f32)
        nc.sync.dma_start(out=w2_sb, in_=w2_d[:].rearrange("k p n -> p k n"))

        # ---------------- Phase A: coords plane + qkv ----------------
        with tc.tile_pool(name="wA", bufs=3) as wa, \
             tc.tile_pool(name="psAtr", bufs=2, space="PSUM") as psatr, \
             tc.tile_pool(name="psAmm", bufs=2, space="PSUM") as psamm:
            for b in range(NB):
                cblk = wa.tile([128, 3], f32, tag="cblk")
                nc.sync.dma_start(out=cblk, in_=coords_d[128 * b:128 * (b + 1), :])
                aug = wa.tile([128, 8], f32, tag="aug")
                nc.gpsimd.tensor_scalar_mul(aug[:, 0:3], cblk, 2.0)
                csq = wa.tile([128, 3], f32, tag="csq")
                nc.vector.tensor_tensor(out=csq, in0=cblk, in1=cblk, op=A.mult)
                sq = wa.tile([128, 1], f32, tag="sq")
                nc.vector.tensor_reduce(out=sq, in_=csq, axis=AX.X, op=A.add)
                # pen = BIG*(mask-1) - |c|^2, computed as (BIG*mask - BIG) - sq
                # so unmasked rows get exactly -sq (no 1e9 cancellation).
                pent = wa.tile([128, 1], f32, tag="pent")
                nc.gpsimd.tensor_scalar(
                    out=pent, in0=maskc[:, b:b + 1], scalar1=float(BIG),
                    scalar2=float(-BIG), op0=A.mult, op1=A.add)
                nc.vector.tensor_sub(out=aug[:, 3:4], in0=pent, in1=sq)
                nc.gpsimd.tensor_copy(out=aug[:, 4:7], in_=cblk)
                nc.gpsimd.memset(aug[:, 7:8], 1.0)
                augT = psatr.tile([8, 128], f32, tag="trA")
                nc.tensor.transpose(augT, aug, ident)
                nc.vector.tensor_copy(out=plane[:, 128 * b:128 * (b + 1)], in_=augT)
                if b < QB:
                    augq = wa.tile([128, 8], f32, tag="augq")
                    nc.gpsimd.tensor_copy(out=augq[:, 0:3], in_=cblk)
                    nc.gpsimd.memset(augq[:, 3:4], 1.0)
                    nc.gpsimd.memset(augq[:, 4:8], 0.0)
                    augqT = psatr.tile([8, 128], f32, tag="trA")
                    nc.tensor.transpose(augqT, augq, ident)
                    nc.vector.tensor_copy(out=lhsTz[:, 128 * b:128 * (b + 1)], in_=augqT)

                xb = wa.tile([128, C], f32, tag="xb")
                nc.sync.dma_start(out=xb, in_=x_d[128 * b:128 * (b + 1), :])
                st6 = wa.tile([128, 6], f32, tag="st6")
                nc.vector.bn_stats(out=st6, in_=xb)
                mv = wa.tile([128, 2], f32, tag="mv")
                nc.vector.bn_aggr(out=mv, in_=st6)
                sd = wa.tile([128, 1], f32, tag="sd")
                nc.scalar.activation(out=sd, in_=mv[:, 1:2], func=AF.Sqrt,
                                     bias=epsT, scale=1.0)
                rstd = wa.tile([128, 1], f32, tag="rstd")
                nc.vector.reciprocal(out=rstd, in_=sd)
                xa = wa.tile([128, C + 2], f32, tag="xa")
                nc.vector.tensor_scalar_mul(xa[:, 0:C], xb, rstd)
                nc.vector.tensor_scalar(
                    out=xa[:, C:C + 1], in0=mv[:, 0:1], scalar1=rstd,
                    scalar2=-1.0, op0=A.mult, op1=A.mult)
                nc.vector.memset(xa[:, C + 1:C + 2], 1.0)
                xaT_p = psatr.tile([C + 2, 128], f32, tag="trA")
                nc.tensor.transpose(xaT_p, xa, ident)
                xaT = wa.tile([C + 2, 128], f32, tag="xaT")
                nc.scalar.copy(out=xaT, in_=xaT_p)
                qkv_p = psamm.tile([128, 3 * C], f32, tag="mmq")
                nc.tensor.matmul(qkv_p, xaT, wqkv_sb, start=True, stop=True)
                if b < QB:
                    nc.scalar.copy(out=q_res[:, b, :], in_=qkv_p[:, 0:C])
                kvb = wa.tile([128, 2 * C], f32, tag="kvb")
                nc.scalar.copy(out=kvb, in_=qkv_p[:, C:3 * C])
                nc.sync.dma_start(out=kv_dram[128 * b:128 * (b + 1), :], in_=kvb)

        tc.strict_bb_all_engine_barrier()

        # ---------------- Phase B+C fused ----------------
        with tc.tile_pool(name="wB", bufs=3) as wb, \
             tc.tile_pool(name="kvp", bufs=3) as kvp, \
             tc.tile_pool(name="wC", bufs=3) as wc, \
             tc.tile_pool(name="rq0", bufs=1, space="PSUM") as rq0, \
             tc.tile_pool(name="rq1", bufs=1, space="PSUM") as rq1, \
             tc.tile_pool(name="psctr", bufs=2, space="PSUM") as psctr, \
             tc.tile_pool(name="pscmm", bufs=2, space="PSUM") as pscmm:
            rqp = [rq0, rq1]
            w0 = rq0.tile([128, 1024], f32, tag="rq")
            nc.vector.memset(w0[:, 0:8], 0.0)
            w1t = rq1.tile([128, 1024], f32, tag="rq")
            nc.vector.memset(w1t[:, 0:8], 0.0)
            for _w in range(2):
                warmC = psctr.tile([128, 128], f32, tag="trC", name=f"warmC{_w}")
                nc.vector.memset(warmC[:, 0:8], 0.0)
                warmM = pscmm.tile([128, MLP], f32, tag="mmC", name=f"warmM{_w}")
                nc.vector.memset(warmM[:, 0:8], 0.0)
            for b in range(QB):
                lhsT = lhsTz[:, 128 * b:128 * (b + 1)]
                cv = wb.tile([128, 128], f32, tag="cv")
                rank_sb = wb.tile([128, N], f32, tag="rank_sb")
                for qt in range(4):
                    rkq = rqp[qt % 2].tile([128, 1024], f32, tag="rq",
                                           name=f"rkq{b}_{qt}")
                    for c2 in range(2):
                        nc.tensor.matmul(
                            rkq[:, 512 * c2:512 * (c2 + 1)], lhsT,
                            plane[:, 1024 * qt + 512 * c2:
                                  1024 * qt + 512 * (c2 + 1)],
                            start=True, stop=True)
                    nc.scalar.copy(out=rank_sb[:, 1024 * qt:1024 * (qt + 1)],
                                   in_=rkq)
                    for c4 in range(4):
                        nc.vector.max(
                            out=cv[:, 32 * qt + 8 * c4:32 * qt + 8 * (c4 + 1)],
                            in_=rkq[:, 256 * c4:256 * (c4 + 1)])
                t8 = wb.tile([128, 8], f32, tag="t8")
                nc.vector.max(out=t8, in_=cv)
                cvm = wb.tile([128, 128], f32, tag="cvm")
                nc.vector.match_replace(out=cvm, in_to_replace=t8, in_values=cv,
                                        imm_value=NEG)
                n8 = wb.tile([128, 8], f32, tag="n8")
                nc.vector.max(out=n8, in_=cvm)
                n2p = wb.tile([128, 8], f32, tag="n2p")
                nc.vector.memset(n2p, NEG)
                nc.vector.tensor_copy(out=n2p[:, 0:2], in_=n8[:, 0:2])

                pos0 = wb.tile([128, 8], u32, tag="pos0")
                nc.vector.max_index(out=pos0, in_max=t8, in_values=rank_sb)
                pos1 = wb.tile([128, 8], u32, tag="pos1")
                nc.vector.max_index(out=pos1, in_max=n2p, in_values=rank_sb)
                idx32 = wb.tile([128, KNN], i32, tag="idx32")
                nc.gpsimd.tensor_copy(out=idx32[:, 0:8], in_=pos0)
                nc.gpsimd.tensor_copy(out=idx32[:, 8:KNN], in_=pos1[:, 0:2])

                # kv gather: one indirect DMA per neighbor slot (one row per
                # partition per call)
                kvn = kvp.tile([128, KNN, 2 * C], f32, tag="kvn")
                for j in range(KNN):
                    nc.gpsimd.indirect_dma_start(
                        out=kvn[:, j, :],
                        out_offset=None,
                        in_=kv_dram[:],
                        in_offset=bass.IndirectOffsetOnAxis(
                            ap=idx32[:, j:j + 1], axis=0),
                    )

                # scores s[i, j, h] = sum_d q*kn   (gpsimd)
                prod = wb.tile([128, KNN, H, DH], f32, tag="prod")
                kn_v = kvn[:, :, 0:C].rearrange("p j (h d) -> p j h d", d=DH)
                qa = q_res[:, b, :]
                q_v = view(qa, 0, [[0, KNN], [DH, H], [1, DH]])
                nc.vector.tensor_tensor(out=prod, in0=kn_v, in1=q_v, op=A.mult)
                s = wb.tile([128, KNN, H], f32, tag="s")
                nc.vector.tensor_reduce(out=s, in_=prod, axis=AX.X, op=A.add)

                pk8 = wb.tile([128, H, 8], f32, tag="pk8")
                for h in range(H):
                    nc.vector.max(out=pk8[:, h, :], in_=s[:, :, h])
                e = wb.tile([128, KNN, H], f32, tag="e")
                nc.scalar.activation(out=e, in_=s, func=AF.Exp, bias=0.0, scale=1.0)
                s_hj = view(s[:], 0, [[1, H], [H, KNN]])
                t4_hj = view(pk8[:], 3, [[8, H], [0, KNN]])
                e_hj = view(e[:], 0, [[1, H], [H, KNN]])
                m = wb.tile([128, H, KNN], f32, tag="m")
                nc.vector.tensor_tensor(out=m, in0=s_hj, in1=t4_hj, op=A.is_ge)
                u = wb.tile([128, H, KNN], f32, tag="u")
                nc.vector.tensor_tensor(out=u, in0=m, in1=e_hj, op=A.mult)
                rs = wb.tile([128, H], f32, tag="rs")
                nc.vector.tensor_reduce(out=rs, in_=u, axis=AX.X, op=A.add)
                rc = wb.tile([128, H], f32, tag="rc")
                nc.vector.reciprocal(out=rc, in_=rs)
                wn = wb.tile([128, H, KNN], f32, tag="wn")
                rc_b = view(rc[:], 0, [[1, H], [0, KNN]])
                nc.vector.tensor_tensor(out=wn, in0=u, in1=rc_b, op=A.mult)
                prod2 = wb.tile([128, H, DH, KNN], f32, tag="p2")
                vn_hdj = view(kvn[:], C, [[DH, H], [1, DH], [2 * C, KNN]])
                wn_hdj = view(wn[:], 0, [[KNN, H], [0, DH], [1, KNN]])
                nc.vector.tensor_tensor(out=prod2, in0=vn_hdj, in1=wn_hdj, op=A.mult)
                ob = wc.tile([128, C], f32, tag="ob")
                o_v = ob[:].rearrange("p (h d) -> p h d", d=DH)
                nc.vector.tensor_reduce(out=o_v, in_=prod2, axis=AX.X, op=A.add)

                # ---- C: proj + residual + MLP (fused per block) ----
                oT_p = psctr.tile([128, 128], f32, tag="trC")
                nc.tensor.transpose(oT_p[0:C, :], ob, ident)
                oT = wc.tile([C, 128], f32, tag="oT")
                nc.scalar.copy(out=oT, in_=oT_p[0:C, :])
                mm1 = pscmm.tile([128, MLP], f32, tag="mmC")
                a_p = mm1[:, 0:C]
                nc.tensor.matmul(a_p, oT, wp_sb, start=True, stop=True)
                xb = wc.tile([128, C], f32, tag="xb2")
                nc.sync.dma_start(out=xb, in_=x_d[128 * b:128 * (b + 1), :])
                t1 = wc.tile([128, C], f32, tag="t1")
                nc.vector.tensor_add(out=t1, in0=a_p, in1=hb_b)
                x2 = wc.tile([128, C], f32, tag="x2")
                nc.vector.scalar_tensor_tensor(
                    out=x2, in0=t1, scalar=maskc[:, b:b + 1], in1=xb,
                    op0=A.mult, op1=A.add)

                st6 = wc.tile([128, 6], f32, tag="st6C")
                nc.vector.bn_stats(out=st6, in_=x2)
                mv = wc.tile([128, 2], f32, tag="mvC")
                nc.vector.bn_aggr(out=mv, in_=st6)
                lnv = wc.tile([128, 1], f32, tag="lnv")
                nc.scalar.activation(out=lnv, in_=mv[:, 1:2], func=AF.Ln,
                                     bias=epsT, scale=1.0)
                rstd = wc.tile([128, 1], f32, tag="rstdC")
                nc.scalar.activation(out=rstd, in_=lnv, func=AF.Exp,
                                     bias=0.0, scale=-0.5)
                x2a = wc.tile([128, C + 2], f32, tag="x2a")
                nc.vector.tensor_scalar_mul(x2a[:, 0:C], x2, rstd)
                nc.vector.tensor_scalar(
                    out=x2a[:, C:C + 1], in0=mv[:, 0:1], scalar1=rstd,
                    scalar2=-1.0, op0=A.mult, op1=A.mult)
                nc.vector.memset(x2a[:, C + 1:C + 2], 1.0)
                x2aT_p = psctr.tile([128, 128], f32, tag="trC")
                nc.tensor.transpose(x2aT_p[0:C + 2, :], x2a, ident)
                x2aT = wc.tile([C + 2, 128], f32, tag="x2aT")
                nc.scalar.copy(out=x2aT, in_=x2aT_p[0:C + 2, :])
                mm2 = pscmm.tile([128, MLP], f32, tag="mmC")
                g1_p = mm2[:, :]
                nc.tensor.matmul(g1_p, x2aT, w1_sb, start=True, stop=True)

                # gelu(x) = x * sigmoid(2 * c1 * (x + c0 x^3))
                #         = x * 1/(1 + exp(-2 c1 x (1 + c0 x^2)))
                g1s = wc.tile([128, MLP], f32, tag="g1s")
                nc.scalar.copy(out=g1s, in_=g1_p)
                xg2 = wc.tile([128, MLP], f32, tag="xg2")
                nc.scalar.activation(out=xg2, in_=g1s, func=AF.Square,
                                     bias=0.0, scale=1.0)
                gco = wc.tile([128, MLP], f32, tag="gco")
                nc.gpsimd.tensor_scalar(out=gco, in0=xg2, scalar1=GELU_C0,
                                        scalar2=1.0, op0=A.mult, op1=A.add)
                gf = wc.tile([128, MLP], f32, tag="gf")
                nc.vector.tensor_tensor(out=gf, in0=gco, in1=g1s, op=A.mult)
                em = wc.tile([128, MLP], f32, tag="em")
                nc.scalar.activation(out=em, in_=gf, func=AF.Exp,
                                     bias=0.0, scale=-2.0 * GELU_C1)
                ep1 = wc.tile([128, MLP], f32, tag="ep1")
                nc.gpsimd.tensor_scalar_add(ep1, em, 1.0)
                sg = wc.tile([128, MLP], f32, tag="sg")
                nc.vector.reciprocal(out=sg, in_=ep1)
                gl = wc.tile([128, MLP], f32, tag="gl")
                nc.vector.tensor_tensor(out=gl, in0=sg, in1=g1s, op=A.mult)

                glT = []
                for k in range(2):
                    glT_p = psctr.tile([128, 128], f32, tag="trC")
                    nc.tensor.transpose(glT_p, gl[:, 128 * k:128 * (k + 1)], ident)
                    glTk = wc.tile([128, 128], f32, tag=f"glT{k}")
                    nc.scalar.copy(out=glTk, in_=glT_p)
                    glT.append(glTk)
                mm3 = pscmm.tile([128, MLP], f32, tag="mmC")
                h2_p = mm3[:, 0:C]
                nc.tensor.matmul(h2_p, glT[0], w2_sb[:, 0, :], start=True, stop=False)
                nc.tensor.matmul(h2_p, glT[1], w2_sb[:, 1, :], start=False, stop=True)
                x2b = wc.tile([128, C], f32, tag="x2b")
                nc.vector.tensor_add(out=x2b, in0=x2, in1=hb2_b)
                fin = wc.tile([128, C], f32, tag="fin")
                nc.vector.tensor_add(out=fin, in0=h2_p, in1=x2b)
                nc.sync.dma_start(out=out_d[128 * b:128 * (b + 1), :], in_=fin)

    nc.compile()
    return nc


@functools.cache
def _get_nc():
    return _build_nc()


def _prepare_weights(ln1_g, ln1_b, ln2_g, ln2_b, w_qkv, b_qkv, w_proj, b_proj,
                     w1, b1, w2, b2):
    f = np.float32
    ln1_g, ln1_b = np.asarray(ln1_g, f), np.asarray(ln1_b, f)
    ln2_g, ln2_b = np.asarray(ln2_g, f), np.asarray(ln2_b, f)
    w_qkv, b_qkv = np.asarray(w_qkv, f), np.asarray(b_qkv, f)
    w_proj, b_proj = np.asarray(w_proj, f), np.asarray(b_proj, f)
    w1, b1 = np.asarray(w1, f), np.asarray(b1, f)
    w2, b2 = np.asarray(w2, f), np.asarray(b2, f)

    Wq = ln1_g[:, None] * w_qkv
    u = ln1_g @ w_qkv
    vc = ln1_b @ w_qkv + b_qkv
    Waug = np.concatenate([Wq, u[None], vc[None]], 0).astype(f)
    Waug[:, 0:C] *= f(1.0 / np.sqrt(DH))      # fold attention scale into q
    Wp = (0.5 * w_proj).astype(f)
    hb = (0.5 * b_proj).astype(f)
    W1g = ln2_g[:, None] * w1
    u1 = ln2_g @ w1
    v1 = ln2_b @ w1 + b1
    W1aug = np.concatenate([W1g, u1[None], v1[None]], 0).astype(f)
    W2r = np.ascontiguousarray((0.5 * w2).reshape(2, 128, C)).astype(f)
    hb2 = (0.5 * b2).astype(f)
    return dict(Wqkv=Waug, Wp=Wp, hb=hb, W1=W1aug, W2=W2r, hb2=hb2)


def _prepare_in_maps(inputs, voxel_coords, non_empty_mask, **weights):
    f = np.float32
    x = np.asarray(inputs, f)
    coords = np.asarray(voxel_coords, f)
    mask = np.asarray(non_empty_mask)
    wmaps = _prepare_weights(**weights)
    in_maps = []
    for c in range(8):
        bi, hf = c // 2, c % 2
        rot = lambda a: np.ascontiguousarray(np.roll(a, -QN * hf, axis=0))
        mr = rot(mask[bi]).astype(f)
        in_maps.append(dict(
            x=rot(x[bi]),
            coords=rot(coords[bi]),
            mask_cols=np.ascontiguousarray(mr.reshape(NB, 128).T),
            **wmaps,
        ))
    return in_maps


def _assemble(results):
    out = np.empty((B, N, C), np.float32)
    for c in range(8):
        bi, hf = c // 2, c % 2
        out[bi, QN * hf:QN * (hf + 1)] = results[c]["out"]
    return out


def kernel(inputs, voxel_coords, non_empty_mask, ln1_g, ln1_b, ln2_g, ln2_b,
           w_qkv, b_qkv, w_proj, b_proj, w1, b1, w2, b2):
    from concourse.bass_utils import run_bass_kernel_spmd

    nc = _get_nc()
    in_maps = _prepare_in_maps(
        inputs, voxel_coords, non_empty_mask,
        ln1_g=ln1_g, ln1_b=ln1_b, ln2_g=ln2_g, ln2_b=ln2_b,
        w_qkv=w_qkv, b_qkv=b_qkv, w_proj=w_proj, b_proj=b_proj,
        w1=w1, b1=b1, w2=w2, b2=b2)
    res = run_bass_kernel_spmd(nc, in_maps, list(range(8)))
    return _assemble(res.results)
